# revision 41
# baseline (speedup 1.0000x reference)
"""Causal multi-head self-attention block (B=2, T=2048, C=1024, H=16) on 8
Trainium2 NeuronCores.

Sharding: core c = 4*b + g handles batch b (2-way data parallel) and head
group g (4-way tensor parallel over the 16 heads -> 4 heads/core).
c_attn is column-sharded (each core computes K/Q/V features only for its 4
heads); c_proj is row-sharded (each core contracts its 4 heads' attn output
against the matching w_proj columns and emits a full-width partial output).
The 4 partial outputs per batch are summed on the host (+ b_proj).

Per-core device pipeline (all matmuls bf16 with fp32 PSUM accumulation):
  1. KQ^T = (w_kq x)        -> [feat, T] layout, feat on partitions
  2. V    = (x^T w_v^T)     -> [T, d] natural layout, augmented with a
     ones column so the AV matmul also yields the softmax denominators
  3. per head pair, per 512-wide q chunk, over live (causal) k tiles:
       aff^T[k,q] = K^T.T Q^T   (two heads row-packed in the PE array)
       E = exp(0.125*aff^T)     (ScalarE, cast bf16; diagonal tiles masked)
       [attn^T unnorm; sums] += V_aug.T E   (M=65, per head)
     then r = 1/sums (DVE), broadcast across partitions via a K=33 fp32
     selector matmul, normalize on DVE.
  4. out_partial = attn^T.T w_proj_slice -> [T, C] natural, DMA to HBM.

Because each engine executes its compiled instruction stream strictly in
order, emission order is scheduling: the AV matmul of tile i is emitted
after the aff matmul of tile i+1 (hides the ScalarE exp latency), block
epilogues (reciprocal/broadcast/normalize) are deferred into the next
block, and projection chunks are emitted inside later attention blocks.
"""

import os
import sys

for _p in ("/opt/trn_rl_repo",):
    if os.path.isdir(_p) and _p not in sys.path:
        sys.path.append(_p)

import numpy as np
import ml_dtypes

B, T, C, H, D = 2, 2048, 1024, 16, 64
N_CORES = 8
HPC = H // 4          # heads per core = 4
CPC = HPC * D         # attn feature cols per core = 256
KQF = 2 * CPC         # K+Q features per core = 512
TCH = 512             # q-chunk width
NJ = T // TCH         # 4 q chunks
NTI = T // 128        # 16 t tiles

_CACHE = {}


def _build_program():
    from contextlib import ExitStack

    import concourse.bass as bass
    import concourse.mybir as mybir
    import concourse.tile as tile
    from concourse import bacc
    from concourse.bass import ts

    f32 = mybir.dt.float32
    f32r = mybir.dt.float32r
    bf16 = mybir.dt.bfloat16
    Exp = mybir.ActivationFunctionType.Exp

    nc = bacc.Bacc("TRN2", target_bir_lowering=False, debug=False,
                   num_devices=N_CORES)

    debug_dump = bool(os.environ.get("KERNEL_DEBUG_DUMP"))
    xT_d = nc.dram_tensor("xT", [128, 8, T], bf16, kind="ExternalInput")
    wkq_d = nc.dram_tensor("wkq", [128, 8, KQF], bf16, kind="ExternalInput")
    bkq_d = nc.dram_tensor("bkq", [128, 4], f32, kind="ExternalInput")
    wv_d = nc.dram_tensor("wv", [128, 8, CPC], bf16, kind="ExternalInput")
    bv_d = nc.dram_tensor("bv", [1, CPC], bf16, kind="ExternalInput")
    wp_d = nc.dram_tensor("wp", [128, 2, C], bf16, kind="ExternalInput")
    mask_d = nc.dram_tensor("mask", [128, 4, TCH], bf16, kind="ExternalInput")
    out_d = nc.dram_tensor("out", [T, C], f32, kind="ExternalOutput")
    if debug_dump:
        dbg_kq_d = nc.dram_tensor("dbg_kq", [128, 4, T], f32,
                                  kind="ExternalOutput")
        dbg_v_d = nc.dram_tensor("dbg_v", [128, NTI, HPC, D], f32,
                                 kind="ExternalOutput")
        dbg_attn_d = nc.dram_tensor("dbg_attn", [128, 2, T], f32,
                                    kind="ExternalOutput")

    with tile.TileContext(nc) as tc, ExitStack() as ctx:
        pp = ctx.enter_context(tc.tile_pool(name="persist", bufs=1))
        o_pool = ctx.enter_context(tc.tile_pool(name="outp", bufs=1))
        xT_sb = pp.tile([128, 8, T], bf16)
        wkq_sb = pp.tile([128, 8, KQF], bf16)
        bkq_sb = pp.tile([128, 4], f32)
        wv_sb = pp.tile([128, 8, CPC], bf16)
        bv_sb = pp.tile([1, CPC], bf16)
        wp_sb = pp.tile([128, 2, C], bf16)
        mask_sb = pp.tile([128, 4, TCH], bf16)
        kq_sb = pp.tile([128, 4, T], bf16)
        v_sb = pp.tile([128, NTI, HPC, D + 1], bf16)
        attn_sb = pp.tile([128, 2, T], bf16)
        ones_sb = pp.tile([128, 128], bf16)
        # per-head reciprocal rows at partitions 0 and 32 (engine ops need
        # 32-aligned partition bases); sel2 broadcasts them to partitions
        # 0:64 / 64:128 via one K=33 fp32 matmul
        r2_sb = pp.tile([33, 2 * NJ * TCH], f32)
        r2r_sb = pp.tile([33, 2 * NJ * TCH], f32r)
        sel2_sb = pp.tile([33, 128], f32)
        sel2r_sb = pp.tile([33, 128], f32r)

        for c in range(8):
            nc.sync.dma_start(wkq_sb[:, c, :], wkq_d[:, c, :])
        nc.sync.dma_start(bkq_sb[:], bkq_d[:])
        for c in range(8):
            nc.sync.dma_start(xT_sb[:, c, 0:TCH], xT_d[:, c, 0:TCH])
        nc.sync.dma_start(wv_sb[:], wv_d[:])
        for tch in range(1, NJ):
            nc.sync.dma_start(xT_sb[:, :, ts(tch, TCH)],
                              xT_d[:, :, ts(tch, TCH)])
        nc.sync.dma_start(bv_sb[:], bv_d[:])
        nc.sync.dma_start(wp_sb[:], wp_d[:])
        nc.sync.dma_start(mask_sb[:], mask_d[:])
        nc.any.memset(ones_sb[:], 1.0)
        for ti in range(NTI):
            nc.any.memset(v_sb[:, ti, :, D:D + 1], 1.0)
        nc.any.memset(r2_sb[:], 0.0)
        nc.any.memset(sel2_sb[:], 0.0)
        nc.any.memset(sel2_sb[0:1, 0:64], 1.0)
        nc.any.memset(sel2_sb[32:33, 64:128], 1.0)
        # f32r copies: the PE's fp32r mode needs producers typed f32r; the
        # bits are identical so DVE copies just retype them (runs during
        # the initial DMA window)
        with nc.allow_low_precision(reason="f32r retype, bit-identical"):
            nc.vector.tensor_copy(r2r_sb[:], r2_sb[:])
            nc.vector.tensor_copy(sel2r_sb[:], sel2_sb[:])

        # One shared PSUM pool: 8 banks = acc(2) + aff(3) + work(3).
        pa_pool = ctx.enter_context(
            tc.tile_pool(name="pall", bufs=1, space="PSUM"))
        e_pool = ctx.enter_context(tc.tile_pool(name="epool", bufs=1))
        r_pool = ctx.enter_context(tc.tile_pool(name="rpool", bufs=1))

        def emit_kq_tile(m, tch):
            emit_kq_tiles([m], tch)

        def emit_kq_tiles(ms, tch):
            # interleave the c-loops of several feature tiles so the PE can
            # advance as each 128-row chunk of x arrives from HBM
            pk = {m: pa_pool.tile([128, TCH], f32, tag="work", bufs=2,
                                  name="pkq") for m in ms}
            for c in range(8):
                for m in ms:
                    nc.tensor.matmul(
                        pk[m][:], wkq_sb[:, c, ts(m, 128)],
                        xT_sb[:, c, ts(tch, TCH)],
                        start=(c == 0), stop=(c == 7))
            for m in ms:
                nc.vector.tensor_scalar_add(
                    kq_sb[:, m, ts(tch, TCH)], pk[m][:], bkq_sb[:, m:m + 1])

        def emit_v(tis):
            for ti in tis:
                pv = pa_pool.tile([128, CPC], f32, tag="work", bufs=2,
                                  name="pv")
                for c in range(8):
                    nc.tensor.matmul(
                        pv[:], xT_sb[:, c, ts(ti, 128)], wv_sb[:, c, :],
                        start=(c == 0), stop=False)
                nc.tensor.matmul(pv[:], ones_sb[0:1, :], bv_sb[:],
                                 start=False, stop=True)
                nc.vector.tensor_copy(
                    v_sb[:, ti, :, 0:D],
                    pv[:].rearrange("p (h d) -> p h d", h=HPC))

        def emit_attn_block(g, j, hooks=()):
            """Emit one (head-pair, q-chunk) attention block.

            Returns a `finalize` closure (reciprocal + broadcast +
            normalize) that the caller schedules later, typically inside
            the next block, so the PE never waits on it.
            """
            pav0 = pa_pool.tile([128, TCH], f32, tag="acc", bufs=2,
                                name="pav0")
            pav1 = pa_pool.tile([128, TCH], f32, tag="acc", bufs=2,
                                name="pav1")
            n_live = 4 * j + 4
            es = {}

            def emit_aff(i):
                # diagonal tiles only touch queries q >= k: narrow the
                # q-range to [q0:TCH] and mask just its first 128 columns
                q0 = max(0, 128 * i - TCH * j)
                qsl = slice(j * TCH + q0, (j + 1) * TCH)
                a0 = pa_pool.tile([128, TCH], f32, tag="aff", bufs=4,
                                  name="a0")
                a1 = pa_pool.tile([128, TCH], f32, tag="aff", bufs=4,
                                  name="a1")
                nc.tensor.matmul(
                    a0[:, q0:], kq_sb[0:64, g, ts(i, 128)],
                    kq_sb[0:64, 2 + g, qsl], start=True, stop=True)
                nc.tensor.matmul(
                    a1[:, q0:], kq_sb[64:128, g, ts(i, 128)],
                    kq_sb[64:128, 2 + g, qsl], start=True, stop=True)
                e0 = e_pool.tile([128, TCH], bf16, tag="e0", bufs=4,
                                 name="e0")
                e1 = e_pool.tile([128, TCH], bf16, tag="e1", bufs=4,
                                 name="e1")
                nc.scalar.activation(e0[:, q0:], a0[:, q0:], Exp,
                                     scale=0.125)
                nc.scalar.activation(e1[:, q0:], a1[:, q0:], Exp,
                                     scale=0.125)
                if q0 > 0 or i == 4 * j:
                    tri = mask_sb[:, 0, 0:128]
                    nc.vector.tensor_mul(e0[:, q0:q0 + 128],
                                         e0[:, q0:q0 + 128], tri)
                    nc.vector.tensor_mul(e1[:, q0:q0 + 128],
                                         e1[:, q0:q0 + 128], tri)
                es[i] = (e0, e1, q0)

            def emit_av(i):
                e0, e1, q0 = es.pop(i)
                first, last = (i == 0), (i == n_live - 1)
                nc.tensor.matmul(
                    pav0[0:65, q0:], v_sb[:, i, 2 * g + 0, :], e0[:, q0:],
                    start=first, stop=last)
                nc.tensor.matmul(
                    pav1[0:65, q0:], v_sb[:, i, 2 * g + 1, :], e1[:, q0:],
                    start=first, stop=last)

            for i in range(n_live):
                emit_aff(i)
                if i == 1:
                    for h in hooks:
                        h()
                if i >= 2:
                    emit_av(i - 2)
            emit_av(n_live - 2)
            emit_av(n_live - 1)

            def finalize():
                r2g = r2r_sb[:, ts(g * NJ + j, TCH)]
                with nc.allow_low_precision(reason="f32r recip rows"):
                    nc.vector.reciprocal(r2g[0:1, :], pav0[64:65, :])
                    nc.vector.reciprocal(r2g[32:33, :], pav1[64:65, :])
                pr = pa_pool.tile([128, TCH], f32, tag="work", bufs=2,
                                  name="pr")
                nc.tensor.matmul(pr[:], sel2r_sb[:], r2g[:],
                                 start=True, stop=True)
                rb = r_pool.tile([128, TCH], f32, tag="rb", bufs=3)
                nc.vector.tensor_copy(rb[:], pr[:])
                nc.vector.tensor_mul(
                    attn_sb[0:64, g, ts(j, TCH)], pav0[0:64, :],
                    rb[0:64, :])
                nc.vector.tensor_mul(
                    attn_sb[64:128, g, ts(j, TCH)], pav1[0:64, :],
                    rb[64:128, :])

            return finalize

        def emit_proj(ti):
            for och in range(2):
                po = pa_pool.tile([128, 512], f32, tag="work", bufs=2,
                                  name="po")
                nc.tensor.matmul(
                    po[:], attn_sb[:, 0, ts(ti, 128)],
                    wp_sb[:, 0, ts(och, 512)], start=True, stop=False)
                nc.tensor.matmul(
                    po[:], attn_sb[:, 1, ts(ti, 128)],
                    wp_sb[:, 1, ts(och, 512)], start=False, stop=True)
                ot = o_pool.tile([128, 512], f32, tag="ot", bufs=4)
                nc.vector.tensor_copy(ot[:], po[:])
                nc.sync.dma_start(out_d[ts(ti, 128), ts(och, 512)], ot[:])

        def proj_chunk(j):
            def h():
                for ti in range(4 * j, 4 * j + 4):
                    emit_proj(ti)
            return h

        # schedule: tch-major projections first (feeds from the earliest
        # DMA chunks), then attention g0 with K/Q for heads 2-3 as filler,
        # then attention g1 with output projection as filler.
        fin = None
        for tch in range(NJ):
            emit_kq_tile(0, tch)
            emit_kq_tile(2, tch)
            emit_v(range(4 * tch, 4 * tch + 4))
            hooks = [fin] if fin else []
            fin = emit_attn_block(0, tch, hooks=hooks)
            emit_kq_tile(1, tch)
            emit_kq_tile(3, tch)
        j_order = list(range(NJ))
        prev_j = None
        for j in j_order:
            hooks = [fin]
            if prev_j is not None:
                hooks.append(proj_chunk(prev_j))
            fin = emit_attn_block(1, j, hooks=hooks)
            prev_j = j
        fin()
        proj_chunk(j_order[-1])()

        if debug_dump:
            dbg_pool = ctx.enter_context(tc.tile_pool(name="dbgp", bufs=2))
            for mm in range(4):
                dt_ = dbg_pool.tile([128, T], f32, tag="dkq")
                nc.any.tensor_copy(dt_[:], kq_sb[:, mm, :])
                nc.sync.dma_start(dbg_kq_d[:, mm, :], dt_[:])
            for ti in range(NTI):
                dv = dbg_pool.tile([128, HPC * D], f32, tag="dv")
                nc.any.tensor_copy(
                    dv[:].rearrange("p (h d) -> p h d", h=HPC),
                    v_sb[:, ti, :, 0:D])
                nc.sync.dma_start(
                    dbg_v_d[:, ti].rearrange("p h d -> p (h d)"), dv[:])
            for ct in range(2):
                da = dbg_pool.tile([128, T], f32, tag="dat")
                nc.any.tensor_copy(da[:], attn_sb[:, ct, :])
                nc.sync.dma_start(dbg_attn_d[:, ct, :], da[:])

    nc.compile()
    return nc


def _get_program():
    if "nc" not in _CACHE:
        _CACHE["nc"] = _build_program()
    return _CACHE["nc"]


def _host_mask():
    # mask[off][i, jj] = 1.0 iff key (off*128 + i) <= query jj, for the 4
    # diagonal-overlapping 128x512 tile offsets
    i = np.arange(128)[:, None]
    jj = np.arange(TCH)[None, :]
    m = np.stack([(i + off * 128 <= jj) for off in range(4)], axis=0)
    return m.astype(ml_dtypes.bfloat16)


def _shard_inputs(x, w_attn, b_attn, w_proj, b_proj):
    bf = ml_dtypes.bfloat16
    mask = np.ascontiguousarray(_host_mask().transpose(1, 0, 2))  # (128,4,512)
    in_maps = []
    for c in range(N_CORES):
        b, g = divmod(c, 4)
        hs = slice(g * CPC, (g + 1) * CPC)
        # xT: (C, T) -> (128, 8, T)
        xT = np.ascontiguousarray(
            x[b].T.reshape(8, 128, T).transpose(1, 0, 2)).astype(bf)
        # K block rows 0:C, Q rows C:2C, V rows 2C:3C of w_attn
        wkq = np.concatenate([w_attn[0 + g * CPC:0 + (g + 1) * CPC],
                              w_attn[C + g * CPC:C + (g + 1) * CPC]], axis=0)
        # (KQF, C) -> transpose -> (C, KQF) -> (128, 8, KQF)
        wkq = np.ascontiguousarray(
            wkq.T.reshape(8, 128, KQF).transpose(1, 0, 2)).astype(bf)
        bkq = np.concatenate([b_attn[0 + g * CPC:0 + (g + 1) * CPC],
                              b_attn[C + g * CPC:C + (g + 1) * CPC]])
        bkq = np.ascontiguousarray(bkq.reshape(4, 128).T).astype(np.float32)
        wv = w_attn[2 * C + g * CPC:2 * C + (g + 1) * CPC]  # (CPC, C)
        wv = np.ascontiguousarray(
            wv.T.reshape(8, 128, CPC).transpose(1, 0, 2)).astype(bf)
        bv = b_attn[2 * C + g * CPC:2 * C + (g + 1) * CPC]
        bv = np.ascontiguousarray(bv.reshape(1, CPC)).astype(bf)
        wp = w_proj[:, hs].T  # (CPC, C)
        wp = np.ascontiguousarray(
            wp.reshape(2, 128, C).transpose(1, 0, 2)).astype(bf)
        in_maps.append({"xT": xT, "wkq": wkq, "bkq": bkq, "wv": wv,
                        "bv": bv, "wp": wp, "mask": mask})
    return in_maps


def kernel(x, w_attn, b_attn, w_proj, b_proj):
    from concourse.bass_utils import run_bass_kernel_spmd

    nc = _get_program()
    in_maps = _shard_inputs(x, w_attn, b_attn, w_proj, b_proj)
    res = run_bass_kernel_spmd(nc, in_maps, core_ids=list(range(N_CORES)))
    out = np.zeros((B, T, C), dtype=np.float32)
    for c in range(N_CORES):
        b = c // 4
        out[b] += res.results[c]["out"]
    out += b_proj[None, None, :].astype(np.float32)
    return out


# revision 42
# speedup vs baseline: 1.0777x; 1.0777x over previous
"""Causal multi-head self-attention block (B=2, T=2048, C=1024, H=16) on 8
Trainium2 NeuronCores.

Sharding: core c = 4*b + g handles batch b (2-way data parallel) and head
group g (4-way tensor parallel over the 16 heads -> 4 heads/core).
c_attn is column-sharded (each core computes K/Q/V features only for its 4
heads); c_proj is row-sharded (each core contracts its 4 heads' attn output
against the matching w_proj columns and emits a full-width partial output).
The 4 partial outputs per batch are summed on the host (+ b_proj).

Per-core device pipeline (all matmuls bf16 with fp32 PSUM accumulation):
  1. KQ^T = (w_kq x)        -> [feat, T] layout, feat on partitions
  2. V    = (x^T w_v^T)     -> [T, d] natural layout, augmented with a
     ones column so the AV matmul also yields the softmax denominators
  3. per head pair, per 512-wide q chunk, over live (causal) k tiles:
       aff^T[k,q] = K^T.T Q^T   (two heads row-packed in the PE array)
       E = exp(0.125*aff^T)     (ScalarE, cast bf16; diagonal tiles masked)
       [attn^T unnorm; sums] += V_aug.T E   (M=65, per head)
     then r = 1/sums (DVE), broadcast across partitions via a K=33 fp32
     selector matmul, normalize on DVE.
  4. out_partial = attn^T.T w_proj_slice -> [T, C] natural, DMA to HBM.

Because each engine executes its compiled instruction stream strictly in
order, emission order is scheduling: the AV matmul of tile i is emitted
after the aff matmul of tile i+1 (hides the ScalarE exp latency), block
epilogues (reciprocal/broadcast/normalize) are deferred into the next
block, and projection chunks are emitted inside later attention blocks.
"""

import os
import sys

for _p in ("/opt/trn_rl_repo",):
    if os.path.isdir(_p) and _p not in sys.path:
        sys.path.append(_p)

import numpy as np
import ml_dtypes

B, T, C, H, D = 2, 2048, 1024, 16, 64
N_CORES = 8
HPC = H // 4          # heads per core = 4
CPC = HPC * D         # attn feature cols per core = 256
KQF = 2 * CPC         # K+Q features per core = 512
TCH = 512             # q-chunk width
NJ = T // TCH         # 4 q chunks
NTI = T // 128        # 16 t tiles

_CACHE = {}


def _build_program():
    from contextlib import ExitStack

    import concourse.bass as bass
    import concourse.mybir as mybir
    import concourse.tile as tile
    from concourse import bacc
    from concourse.bass import ts

    f32 = mybir.dt.float32
    f32r = mybir.dt.float32r
    bf16 = mybir.dt.bfloat16
    Exp = mybir.ActivationFunctionType.Exp

    nc = bacc.Bacc("TRN2", target_bir_lowering=False, debug=False,
                   num_devices=N_CORES)

    debug_dump = bool(os.environ.get("KERNEL_DEBUG_DUMP"))
    xT_d = nc.dram_tensor("xT", [128, 8, T], bf16, kind="ExternalInput")
    wkq_d = nc.dram_tensor("wkq", [128, 8, KQF], bf16, kind="ExternalInput")
    bkq_d = nc.dram_tensor("bkq", [128, 4], f32, kind="ExternalInput")
    wv_d = nc.dram_tensor("wv", [128, 8, CPC], bf16, kind="ExternalInput")
    wp_d = nc.dram_tensor("wp", [128, 2, C], bf16, kind="ExternalInput")
    mask_d = nc.dram_tensor("mask", [128, 4, TCH], bf16, kind="ExternalInput")
    out_d = nc.dram_tensor("out", [T, C], f32, kind="ExternalOutput")
    if debug_dump:
        dbg_kq_d = nc.dram_tensor("dbg_kq", [128, 4, T], f32,
                                  kind="ExternalOutput")
        dbg_v_d = nc.dram_tensor("dbg_v", [128, NTI, HPC, D], f32,
                                 kind="ExternalOutput")
        dbg_attn_d = nc.dram_tensor("dbg_attn", [128, 2, T], f32,
                                    kind="ExternalOutput")

    with tile.TileContext(nc) as tc, ExitStack() as ctx:
        pp = ctx.enter_context(tc.tile_pool(name="persist", bufs=1))
        o_pool = ctx.enter_context(tc.tile_pool(name="outp", bufs=1))
        xT_sb = pp.tile([128, 8, T], bf16)
        wkq_sb = pp.tile([128, 8, KQF], bf16)
        bkq_sb = pp.tile([128, 4], f32)
        wv_sb = pp.tile([128, 8, CPC], bf16)
        wp_sb = pp.tile([128, 2, C], bf16)
        mask_sb = pp.tile([128, 4, TCH], bf16)
        kq_sb = pp.tile([128, 4, T], bf16)
        v_sb = pp.tile([128, NTI, HPC, D + 1], bf16)
        attn_sb = pp.tile([128, 2, T], bf16)
        # per-head reciprocal rows at partitions 0 and 32 (engine ops need
        # 32-aligned partition bases); sel2 broadcasts them to partitions
        # 0:64 / 64:128 via one K=33 fp32 matmul
        r2_sb = pp.tile([33, 2 * NJ * TCH], f32)
        r2r_sb = pp.tile([33, 2 * NJ * TCH], f32r)
        sel2_sb = pp.tile([33, 128], f32)
        sel2r_sb = pp.tile([33, 128], f32r)

        for c in range(8):
            nc.sync.dma_start(wkq_sb[:, c, :], wkq_d[:, c, :])
        nc.sync.dma_start(bkq_sb[:], bkq_d[:])
        for c in range(8):
            nc.sync.dma_start(xT_sb[:, c, 0:TCH], xT_d[:, c, 0:TCH])
        nc.sync.dma_start(wv_sb[:], wv_d[:])
        for tch in range(1, NJ):
            nc.sync.dma_start(xT_sb[:, :, ts(tch, TCH)],
                              xT_d[:, :, ts(tch, TCH)])
        nc.sync.dma_start(wp_sb[:], wp_d[:])
        nc.sync.dma_start(mask_sb[:], mask_d[:])
        for ti in range(NTI):
            nc.any.memset(v_sb[:, ti, :, D:D + 1], 1.0)
        nc.any.memset(r2_sb[:], 0.0)
        nc.any.memset(sel2_sb[:], 0.0)
        nc.any.memset(sel2_sb[0:1, 0:64], 1.0)
        nc.any.memset(sel2_sb[32:33, 64:128], 1.0)
        # f32r copies: the PE's fp32r mode needs producers typed f32r; the
        # bits are identical so DVE copies just retype them (runs during
        # the initial DMA window)
        with nc.allow_low_precision(reason="f32r retype, bit-identical"):
            nc.vector.tensor_copy(r2r_sb[:], r2_sb[:])
            nc.vector.tensor_copy(sel2r_sb[:], sel2_sb[:])

        # One shared PSUM pool: 8 banks = acc(2) + aff(3) + work(3).
        pa_pool = ctx.enter_context(
            tc.tile_pool(name="pall", bufs=1, space="PSUM"))
        e_pool = ctx.enter_context(tc.tile_pool(name="epool", bufs=1))
        r_pool = ctx.enter_context(tc.tile_pool(name="rpool", bufs=1))

        def emit_kq_tile(m, tch):
            emit_kq_tiles([m], tch)

        def emit_kq_tiles(ms, tch):
            # interleave the c-loops of several feature tiles so the PE can
            # advance as each 128-row chunk of x arrives from HBM
            pk = {m: pa_pool.tile([128, TCH], f32, tag="work", bufs=2,
                                  name="pkq") for m in ms}
            for c in range(8):
                for m in ms:
                    nc.tensor.matmul(
                        pk[m][:], wkq_sb[:, c, ts(m, 128)],
                        xT_sb[:, c, ts(tch, TCH)],
                        start=(c == 0), stop=(c == 7))
            for m in ms:
                nc.vector.tensor_scalar_add(
                    kq_sb[:, m, ts(tch, TCH)], pk[m][:], bkq_sb[:, m:m + 1])

        def emit_v(tis):
            for ti in tis:
                pv = pa_pool.tile([128, CPC], f32, tag="work", bufs=2,
                                  name="pv")
                for c in range(8):
                    nc.tensor.matmul(
                        pv[:], xT_sb[:, c, ts(ti, 128)], wv_sb[:, c, :],
                        start=(c == 0), stop=(c == 7))
                nc.vector.tensor_copy(
                    v_sb[:, ti, :, 0:D],
                    pv[:].rearrange("p (h d) -> p h d", h=HPC))

        def emit_attn_block(g, j, hooks=()):
            """Emit one (head-pair, q-chunk) attention block.

            Returns a `finalize` closure (reciprocal + broadcast +
            normalize) that the caller schedules later, typically inside
            the next block, so the PE never waits on it.
            """
            pav0 = pa_pool.tile([128, TCH], f32, tag="acc", bufs=3,
                                name="pav0")
            pav1 = pa_pool.tile([128, TCH], f32, tag="acc", bufs=3,
                                name="pav1")
            n_live = 4 * j + 4
            es = {}

            def emit_aff(i):
                # diagonal tiles only touch queries q >= k: narrow the
                # q-range to [q0:TCH] and mask just its first 128 columns
                q0 = max(0, 128 * i - TCH * j)
                qsl = slice(j * TCH + q0, (j + 1) * TCH)
                a0 = pa_pool.tile([128, TCH], f32, tag="aff", bufs=3,
                                  name="a0")
                a1 = pa_pool.tile([128, TCH], f32, tag="aff", bufs=3,
                                  name="a1")
                nc.tensor.matmul(
                    a0[:, q0:], kq_sb[0:64, g, ts(i, 128)],
                    kq_sb[0:64, 2 + g, qsl], start=True, stop=True)
                nc.tensor.matmul(
                    a1[:, q0:], kq_sb[64:128, g, ts(i, 128)],
                    kq_sb[64:128, 2 + g, qsl], start=True, stop=True)
                e0 = e_pool.tile([128, TCH], bf16, tag="e0", bufs=4,
                                 name="e0")
                e1 = e_pool.tile([128, TCH], bf16, tag="e1", bufs=4,
                                 name="e1")
                nc.scalar.activation(e0[:, q0:], a0[:, q0:], Exp,
                                     scale=0.125)
                nc.scalar.activation(e1[:, q0:], a1[:, q0:], Exp,
                                     scale=0.125)
                if q0 > 0 or i == 4 * j:
                    tri = mask_sb[:, 0, 0:128]
                    nc.vector.tensor_mul(e0[:, q0:q0 + 128],
                                         e0[:, q0:q0 + 128], tri)
                    nc.vector.tensor_mul(e1[:, q0:q0 + 128],
                                         e1[:, q0:q0 + 128], tri)
                es[i] = (e0, e1, q0)

            def emit_av(i):
                e0, e1, q0 = es.pop(i)
                first, last = (i == 0), (i == n_live - 1)
                nc.tensor.matmul(
                    pav0[0:65, q0:], v_sb[:, i, 2 * g + 0, :], e0[:, q0:],
                    start=first, stop=last)
                nc.tensor.matmul(
                    pav1[0:65, q0:], v_sb[:, i, 2 * g + 1, :], e1[:, q0:],
                    start=first, stop=last)

            for i in range(n_live):
                emit_aff(i)
                if i == 1:
                    for h in hooks:
                        h()
                if i >= 1:
                    emit_av(i - 1)
            emit_av(n_live - 1)

            def finalize():
                r2g = r2r_sb[:, ts(g * NJ + j, TCH)]
                with nc.allow_low_precision(reason="f32r recip rows"):
                    nc.vector.reciprocal(r2g[0:1, :], pav0[64:65, :])
                    nc.vector.reciprocal(r2g[32:33, :], pav1[64:65, :])
                pr = pa_pool.tile([128, TCH], f32, tag="work", bufs=2,
                                  name="pr")
                nc.tensor.matmul(pr[:], sel2r_sb[:], r2g[:],
                                 start=True, stop=True)
                rb = r_pool.tile([128, TCH], f32, tag="rb", bufs=3)
                nc.vector.tensor_copy(rb[:], pr[:])
                nc.vector.tensor_mul(
                    attn_sb[0:64, g, ts(j, TCH)], pav0[0:64, :],
                    rb[0:64, :])
                nc.vector.tensor_mul(
                    attn_sb[64:128, g, ts(j, TCH)], pav1[0:64, :],
                    rb[64:128, :])

            return finalize

        def emit_proj(ti):
            for och in range(2):
                po = pa_pool.tile([128, 512], f32, tag="work", bufs=2,
                                  name="po")
                nc.tensor.matmul(
                    po[:], attn_sb[:, 0, ts(ti, 128)],
                    wp_sb[:, 0, ts(och, 512)], start=True, stop=False)
                nc.tensor.matmul(
                    po[:], attn_sb[:, 1, ts(ti, 128)],
                    wp_sb[:, 1, ts(och, 512)], start=False, stop=True)
                ot = o_pool.tile([128, 512], f32, tag="ot", bufs=4)
                nc.vector.tensor_copy(ot[:], po[:])
                nc.sync.dma_start(out_d[ts(ti, 128), ts(och, 512)], ot[:])

        def proj_chunk(j):
            def h():
                for ti in range(4 * j, 4 * j + 4):
                    emit_proj(ti)
            return h

        # schedule: tch-major projections first (feeds from the earliest
        # DMA chunks), then attention g0 with K/Q for heads 2-3 as filler,
        # then attention g1 with output projection as filler.
        fin = None
        for tch in range(NJ):
            emit_kq_tile(0, tch)
            emit_kq_tile(2, tch)
            emit_v(range(4 * tch, 4 * tch + 4))
            hooks = [fin] if fin else []
            fin = emit_attn_block(0, tch, hooks=hooks)
            emit_kq_tile(1, tch)
            emit_kq_tile(3, tch)
        j_order = list(range(NJ))
        prev_j = None
        for j in j_order:
            hooks = [fin]
            if prev_j is not None:
                hooks.append(proj_chunk(prev_j))
            fin = emit_attn_block(1, j, hooks=hooks)
            prev_j = j
        fin()
        proj_chunk(j_order[-1])()

        if debug_dump:
            dbg_pool = ctx.enter_context(tc.tile_pool(name="dbgp", bufs=2))
            for mm in range(4):
                dt_ = dbg_pool.tile([128, T], f32, tag="dkq")
                nc.any.tensor_copy(dt_[:], kq_sb[:, mm, :])
                nc.sync.dma_start(dbg_kq_d[:, mm, :], dt_[:])
            for ti in range(NTI):
                dv = dbg_pool.tile([128, HPC * D], f32, tag="dv")
                nc.any.tensor_copy(
                    dv[:].rearrange("p (h d) -> p h d", h=HPC),
                    v_sb[:, ti, :, 0:D])
                nc.sync.dma_start(
                    dbg_v_d[:, ti].rearrange("p h d -> p (h d)"), dv[:])
            for ct in range(2):
                da = dbg_pool.tile([128, T], f32, tag="dat")
                nc.any.tensor_copy(da[:], attn_sb[:, ct, :])
                nc.sync.dma_start(dbg_attn_d[:, ct, :], da[:])

    nc.compile()
    return nc


def _get_program():
    if "nc" not in _CACHE:
        _CACHE["nc"] = _build_program()
    return _CACHE["nc"]


def _host_mask():
    # mask[off][i, jj] = 1.0 iff key (off*128 + i) <= query jj, for the 4
    # diagonal-overlapping 128x512 tile offsets
    i = np.arange(128)[:, None]
    jj = np.arange(TCH)[None, :]
    m = np.stack([(i + off * 128 <= jj) for off in range(4)], axis=0)
    return m.astype(ml_dtypes.bfloat16)


def _shard_inputs(x, w_attn, b_attn, w_proj, b_proj):
    bf = ml_dtypes.bfloat16
    mask = np.ascontiguousarray(_host_mask().transpose(1, 0, 2))  # (128,4,512)
    in_maps = []
    for c in range(N_CORES):
        b, g = divmod(c, 4)
        hs = slice(g * CPC, (g + 1) * CPC)
        # xT: (C, T) -> (128, 8, T)
        xT = np.ascontiguousarray(
            x[b].T.reshape(8, 128, T).transpose(1, 0, 2)).astype(bf)
        # K block rows 0:C, Q rows C:2C, V rows 2C:3C of w_attn
        wkq = np.concatenate([w_attn[0 + g * CPC:0 + (g + 1) * CPC],
                              w_attn[C + g * CPC:C + (g + 1) * CPC]], axis=0)
        # (KQF, C) -> transpose -> (C, KQF) -> (128, 8, KQF)
        wkq = np.ascontiguousarray(
            wkq.T.reshape(8, 128, KQF).transpose(1, 0, 2)).astype(bf)
        bkq = np.concatenate([b_attn[0 + g * CPC:0 + (g + 1) * CPC],
                              b_attn[C + g * CPC:C + (g + 1) * CPC]])
        bkq = np.ascontiguousarray(bkq.reshape(4, 128).T).astype(np.float32)
        wv = w_attn[2 * C + g * CPC:2 * C + (g + 1) * CPC]  # (CPC, C)
        wv = np.ascontiguousarray(
            wv.T.reshape(8, 128, CPC).transpose(1, 0, 2)).astype(bf)
        wp = w_proj[:, hs].T  # (CPC, C)
        wp = np.ascontiguousarray(
            wp.reshape(2, 128, C).transpose(1, 0, 2)).astype(bf)
        in_maps.append({"xT": xT, "wkq": wkq, "bkq": bkq, "wv": wv,
                        "wp": wp, "mask": mask})
    return in_maps


def kernel(x, w_attn, b_attn, w_proj, b_proj):
    from concourse.bass_utils import run_bass_kernel_spmd

    nc = _get_program()
    in_maps = _shard_inputs(x, w_attn, b_attn, w_proj, b_proj)
    res = run_bass_kernel_spmd(nc, in_maps, core_ids=list(range(N_CORES)))
    out = np.zeros((B, T, C), dtype=np.float32)
    for c in range(N_CORES):
        b = c // 4
        out[b] += res.results[c]["out"]
    # V-bias contribution folded out of the device kernel:
    # (attn + bv)^T @ wp  =  attn^T @ wp  +  (bv @ wp)
    bv_full = b_attn[2 * C:3 * C].astype(np.float64)
    bias_out = bv_full @ w_proj.T.astype(np.float64)
    out += (b_proj.astype(np.float64) + bias_out)[None, None, :].astype(
        np.float32)
    return out
